# revision 1
# baseline (speedup 1.0000x reference)
"""GATv2 GraphEncoder kernel for 8 Trainium2 NeuronCores.

Strategy (dst-tile sharding):
  - Sort edges by destination node. Partition the 128-node destination tiles
    contiguously across 8 cores; each core owns its dst-node range and the
    edges flowing into it.
  - Per layer: each core computes xl/xr for its own node slice, AllGathers xl
    (gathers need arbitrary src rows), then runs an edge-parallel pass:
    gather xl[src] and xr[dst] per edge slot via gpsimd dma_gather (int16
    indices, node table split in two halves to fit int16), edge transform ee
    via PE matmul, logits on DVE/ACT, and segment softmax-sum via one-hot
    matmuls into PSUM (out = sum(ex*(xl+xr))/sum(ex) - xr, using the identity
    sum(ex*xr[dst]) = xr*den per node).
  - Self loops ride as an extra chunk per tile whose one-hot is the identity
    (via host dstloc data) and whose edge transform uses the on-device
    computed mean edge_attr; their xl/xr slots are filled by plain DMA.
  - Sum pooling per graph via one-hot matmuls, AllReduce partials, then the
    MLP+LayerNorm replicated on every core.
"""

import math

import numpy as np

TILE = 128
GRP = 4           # chunks per PSUM group (4*128 = 512 = one PSUM bank of f32)
B_STAB = 12.0     # constant softmax shift; logits measured in [-9, 9]
SLOPE = 0.2
G_FIXED = 512     # number of graphs (reference hardcodes 512)
DEBUG_TAPS = False
I16_MAX = 32000   # max rows addressable by one int16 gather table


class Cfg:
    def __init__(self, N, E, FIN, ED, HID, HEADS, G, MH, MO, n_cores, KA, KB):
        self.N, self.E, self.FIN, self.ED = N, E, FIN, ED
        self.HID, self.HEADS, self.G, self.MH, self.MO = HID, HEADS, G, MH, MO
        self.D = HID // HEADS
        self.NC = n_cores
        self.TG = math.ceil(N / TILE)             # global dst tiles
        self.TPC = math.ceil(self.TG / n_cores)   # tiles per core
        self.NPC = self.TPC * TILE                # nodes per core
        self.NPAD = self.NC * self.NPC
        self.KA, self.KB = KA, KB                 # chunks per src-half
        self.KE = KA + KB                         # real-edge chunks per tile
        self.KC = self.KE + 1                     # + self chunk
        self.NGRP = math.ceil(self.KC / GRP)
        self.GT = math.ceil(G / TILE)             # graph tiles for pooling
        self.HALF = self.NPAD // 2 if self.NPAD > I16_MAX else self.NPAD


def _bcast(v, rows=TILE):
    v = np.asarray(v, np.float32).reshape(1, -1)
    return np.broadcast_to(v, (rows, v.shape[1])).copy()


def _wrap16(idx):
    """[n] int -> [128, n//16] int16, wrapped in 16 partitions, replicated."""
    n = idx.shape[0]
    assert n % 16 == 0
    w = idx.reshape(n // 16, 16).T.astype(np.int16)
    return np.tile(w, (8, 1))


def prepare(inputs, n_cores=8):
    """Host-side sharding: returns (cfg, shared inputs, per-core inputs)."""
    x = np.ascontiguousarray(np.asarray(inputs["x"], np.float32))
    ei = np.asarray(inputs["edge_index"]).astype(np.int64)
    ea = np.ascontiguousarray(np.asarray(inputs["edge_attr"], np.float32))
    batch = np.asarray(inputs["batch"]).astype(np.int64)
    N, FIN = x.shape
    E, ED = ea.shape
    HID = inputs["Wl0"].shape[1]
    HEADS = inputs["att0"].shape[0]
    MH = inputs["mW1"].shape[1]
    MO = inputs["mW2"].shape[1]
    G = G_FIXED

    TG = math.ceil(N / TILE)
    TPC = math.ceil(TG / n_cores)
    NPC = TPC * TILE
    NPAD = n_cores * NPC
    HALF = NPAD // 2 if NPAD > I16_MAX else NPAD

    src = ei[0]
    dst = ei[1]
    # sort by (dst tile, src half) => per-tile contiguous, A before B
    tile_of = (dst // TILE).astype(np.int64)
    half_of = (src >= HALF).astype(np.int64)
    order = np.lexsort((half_of, tile_of))
    src_s = src[order].astype(np.int32)
    dst_s = dst[order].astype(np.int32)
    half_s = half_of[order]
    ea_s = np.asarray(ea)[order]
    tile_s = tile_of[order]

    # per-tile counts split by half
    cntA = np.bincount(tile_s[half_s == 0], minlength=TG)
    cntB = np.bincount(tile_s[half_s == 1], minlength=TG)
    KA = int(math.ceil(cntA.max() / TILE)) if cntA.max() > 0 else 0
    KB = int(math.ceil(cntB.max() / TILE)) if cntB.max() > 0 else 0
    cfg = Cfg(N, E, FIN, ED, HID, HEADS, G, MH, MO, n_cores, KA, KB)
    KC, KE = cfg.KC, cfg.KE

    cnt_t = cntA + cntB
    startsT = np.zeros(TG + 1, np.int64)
    np.cumsum(cnt_t, out=startsT[1:])

    shared = {}
    per_core = [dict() for _ in range(n_cores)]

    for c in range(n_cores):
        dstloc = np.full((TILE, TPC * KC), 999.0, np.float32)
        ea_T = np.zeros((ED, TPC * KC * TILE), np.float32)
        ea_em = np.zeros((TILE, TPC * KE * (ED + 1)), np.float32)
        batchg = np.full((TILE, TPC), 99999.0, np.float32)
        xT = np.zeros((FIN, NPC), np.float32)
        gxa = np.zeros((TILE, TPC * KA * 8), np.int16)
        gxb = np.zeros((TILE, TPC * KB * 8), np.int16)
        gxr = np.zeros((TILE, TPC * KE * 8), np.int16)

        nb = c * NPC
        nhi = min(N, nb + NPC)
        if nhi > nb:
            xT[:, : nhi - nb] = x[nb:nhi].T

        for tl in range(TPC):
            tg = c * TPC + tl
            if tg >= TG:
                continue
            e0 = int(startsT[tg])
            nA = int(cntA[tg]); nB = int(cntB[tg])
            idxA = np.zeros(KA * TILE, np.int32)
            idxB = np.zeros(KB * TILE, np.int32)
            idxR = np.zeros(KE * TILE, np.int32)

            sA = src_s[e0 : e0 + nA]
            sB = src_s[e0 + nA : e0 + nA + nB] - HALF
            dA = dst_s[e0 : e0 + nA] - nb
            dB = dst_s[e0 + nA : e0 + nA + nB] - nb
            idxA[:nA] = sA
            idxB[:nB] = sB
            idxR[:nA] = dA
            idxR[KA * TILE : KA * TILE + nB] = dB
            gxa[:, tl * KA * 8 : (tl + 1) * KA * 8] = _wrap16(idxA)
            gxb[:, tl * KB * 8 : (tl + 1) * KB * 8] = _wrap16(idxB)
            gxr[:, tl * KE * 8 : (tl + 1) * KE * 8] = _wrap16(idxR)

            # slot u (within tile) -> chunk u//128, lane u%128
            kA = np.arange(nA); kB_ = np.arange(nB)
            colA = tl * KC + kA // TILE
            colB = tl * KC + KA + kB_ // TILE
            lnA = kA % TILE; lnB = kB_ % TILE
            dstloc[lnA, colA] = (dst_s[e0 : e0 + nA] % TILE).astype(np.float32)
            dstloc[lnB, colB] = (
                dst_s[e0 + nA : e0 + nA + nB] % TILE).astype(np.float32)
            ea_T[:, colA * TILE + lnA] = ea_s[e0 : e0 + nA].T
            ea_T[:, colB * TILE + lnB] = ea_s[e0 + nA : e0 + nA + nB].T
            emA = tl * KE * (ED + 1) + (kA // TILE) * (ED + 1)
            emB = tl * KE * (ED + 1) + (KA + kB_ // TILE) * (ED + 1)
            for f in range(ED):
                ea_em[lnA, emA + f] = ea_s[e0 : e0 + nA, f]
                ea_em[lnB, emB + f] = ea_s[e0 + nA : e0 + nA + nB, f]
            ea_em[lnA, emA + ED] = 1.0
            ea_em[lnB, emB + ED] = 1.0

            n_valid = min(TILE, N - tg * TILE)
            p = np.arange(n_valid)
            dstloc[p, tl * KC + KE] = p.astype(np.float32)
            batchg[p, tl] = batch[tg * TILE : tg * TILE + n_valid].astype(
                np.float32)

        d = per_core[c]
        d["dstloc"] = dstloc
        d["ea_T"] = ea_T
        d["ea_em"] = ea_em
        d["batchg"] = batchg
        d["xT"] = xT
        if KA > 0:
            d["gxa"] = gxa
        if KB > 0:
            d["gxb"] = gxb
        d["gxr"] = gxr

    # ---- shared weight/constant inputs ----------------------------------
    for l in range(3):
        shared[f"Wl{l}"] = np.asarray(inputs[f"Wl{l}"], np.float32)
        shared[f"Wr{l}"] = np.asarray(inputs[f"Wr{l}"], np.float32)
        shared[f"We{l}"] = np.asarray(inputs[f"We{l}"], np.float32)
        att = np.asarray(inputs[f"att{l}"], np.float32).reshape(-1)
        shared[f"attb{l}"] = _bcast(np.tile(att, GRP))        # [128, GRP*HID]
        shared[f"blb{l}"] = _bcast(inputs[f"bl{l}"])
        shared[f"brb{l}"] = _bcast(inputs[f"br{l}"])
        shared[f"outb{l}"] = _bcast(inputs[f"b{l}"])
    shared["iota"] = _bcast(np.arange(TILE, dtype=np.float32))
    shared["iota_g"] = _bcast(np.arange(cfg.GT * TILE, dtype=np.float32))
    shared["ident"] = np.eye(TILE, dtype=np.float32)
    shared["mW1"] = np.asarray(inputs["mW1"], np.float32)
    shared["mb1b"] = _bcast(inputs["mb1"])
    shared["ln_gb"] = _bcast(inputs["ln_g"])
    shared["ln_bb"] = _bcast(inputs["ln_b"])
    mW2 = np.asarray(inputs["mW2"], np.float32)
    mW2t = np.concatenate(
        [mW2[k * TILE : (k + 1) * TILE] for k in range(cfg.MH // TILE)], axis=1
    )
    shared["mW2t"] = np.ascontiguousarray(mW2t)
    shared["mb2b"] = _bcast(inputs["mb2"])

    return cfg, shared, per_core


def build(cfg):
    import concourse.bass as bass
    import concourse.mybir as mybir
    from concourse.bacc import Bacc
    from concourse.tile import TileContext

    F32 = mybir.dt.float32
    I16 = mybir.dt.int16
    AX = mybir.AxisListType
    OP = mybir.AluOpType
    AF = mybir.ActivationFunctionType

    TPC, KC, KE, KA, KB = cfg.TPC, cfg.KC, cfg.KE, cfg.KA, cfg.KB
    NGRP = cfg.NGRP
    HID, ED, FIN, HEADS = cfg.HID, cfg.ED, cfg.FIN, cfg.HEADS
    D = cfg.D
    NPC, NPAD, GT, MH, MO = cfg.NPC, cfg.NPAD, cfg.GT, cfg.MH, cfg.MO
    HALF = cfg.HALF
    CW = 8 + HID  # scatter rhs width per chunk: [ex(8) | ex*m_pre(HID)]

    nc = Bacc(debug=False)

    # ---------------- DRAM I/O ----------------
    din = {}
    def ein(name, shape, dtype=F32):
        din[name] = nc.dram_tensor(name, shape, dtype, kind="ExternalInput")
        return din[name]

    ein("dstloc", [TILE, TPC * KC])
    ein("ea_T", [ED, TPC * KC * TILE])
    ein("ea_em", [TILE, TPC * KE * (ED + 1)])
    ein("batchg", [TILE, TPC])
    ein("xT", [FIN, NPC])
    if KA > 0:
        ein("gxa", [TILE, TPC * KA * 8], I16)
    if KB > 0:
        ein("gxb", [TILE, TPC * KB * 8], I16)
    ein("gxr", [TILE, TPC * KE * 8], I16)
    for l in range(3):
        kin = FIN if l == 0 else HID
        ein(f"Wl{l}", [kin, HID]); ein(f"Wr{l}", [kin, HID]); ein(f"We{l}", [ED, HID])
        ein(f"attb{l}", [TILE, GRP * HID])
        ein(f"blb{l}", [TILE, HID]); ein(f"brb{l}", [TILE, HID]); ein(f"outb{l}", [TILE, HID])
    ein("iota", [TILE, TILE]); ein("iota_g", [TILE, GT * TILE]); ein("ident", [TILE, TILE])
    ein("mW1", [HID, MH]); ein("mb1b", [TILE, MH])
    ein("ln_gb", [TILE, MH]); ein("ln_bb", [TILE, MH])
    ein("mW2t", [TILE, (MH // TILE) * MO]); ein("mb2b", [TILE, MO])

    out_t = nc.dram_tensor("out", [cfg.G, MO], F32, kind="ExternalOutput")
    dbg = {}
    if DEBUG_TAPS:
        for nm, shp in [("d_xl0", [NPC, HID]), ("d_xr0", [NPC, HID]),
                        ("d_xlf0", [NPAD, HID]), ("d_laT", [ED, TPC * TILE]),
                        ("d_h0T", [HID, NPC]), ("d_pool", [GT * TILE, HID]),
                        ("d_poolf", [GT * TILE, HID])]:
            dbg[nm] = nc.dram_tensor(nm, shp, F32, kind="ExternalOutput")

    xl_loc = [nc.dram_tensor(f"xl_loc{l}", [NPC, HID], F32) for l in range(3)]
    xr_loc = [nc.dram_tensor(f"xr_loc{l}", [NPC, HID], F32) for l in range(3)]
    xl_full = [nc.dram_tensor(f"xl_full{l}", [NPAD, HID], F32) for l in range(3)]
    hT_loc = nc.dram_tensor("hT_loc", [HID, NPC], F32)
    laT_loc = nc.dram_tensor("laT_loc", [ED, TPC * TILE], F32)
    pool_part = nc.dram_tensor("pool_part", [GT * TILE, HID], F32)
    pool_full = nc.dram_tensor("pool_full", [GT * TILE, HID], F32)

    RG = [list(range(cfg.NC))]

    with TileContext(nc) as tc:
        with (
            tc.tile_pool(name="const", bufs=1) as cp,
            tc.tile_pool(name="res", bufs=1) as rp,
            tc.tile_pool(name="stream", bufs=2) as sp,
            tc.tile_pool(name="small", bufs=2) as mp,
            tc.tile_pool(name="ps", bufs=8, space="PSUM") as pp,
        ):
            # ---- resident constants ----
            C = {}
            for name in ["iota", "iota_g", "ident", "mW1", "mb1b", "ln_gb",
                         "ln_bb", "mW2t", "mb2b", "batchg"]:
                C[name] = cp.tile(list(din[name].shape), din[name].dtype,
                                  tag=name, name="c_" + name)
                nc.sync.dma_start(out=C[name][:], in_=din[name][:, :])
            for l in range(3):
                for w in [f"Wl{l}", f"Wr{l}", f"We{l}", f"attb{l}", f"blb{l}",
                          f"brb{l}", f"outb{l}"]:
                    C[w] = cp.tile(list(din[w].shape), F32, tag=w, name="c_" + w)
                    nc.sync.dma_start(out=C[w][:], in_=din[w][:, :])

            pool_acc = rp.tile([TILE, GT * HID], F32, tag="poolacc")

            negb = cp.tile([TILE, 1], F32, name="negb")
            nc.vector.memset(negb[:], -B_STAB)
            epsb = cp.tile([TILE, 1], F32, name="epsb")
            nc.vector.memset(epsb[:], 1e-5)

            # ---------------- helpers ----------------
            def build_w(t, w_t):
                dl = mp.tile([TILE, KC], F32, tag="dl")
                nc.sync.dma_start(out=dl[:],
                                  in_=din["dstloc"][:, t * KC : (t + 1) * KC])
                in0 = dl[:].rearrange("p (k o) -> p k o", o=1).to_broadcast(
                    [TILE, KC, TILE])
                in1 = C["iota"][:].rearrange("p (o n) -> p o n", o=1).to_broadcast(
                    [TILE, KC, TILE])
                nc.vector.tensor_tensor(
                    out=w_t[:].rearrange("p (k n) -> p k n", k=KC),
                    in0=in0, in1=in1, op=OP.is_equal)

            def transpose_to(dst_sb, src_sb):
                trp = pp.tile([dst_sb.shape[0], src_sb.shape[0]], F32, tag="ps")
                nc.tensor.transpose(out=trp[:], in_=src_sb, identity=C["ident"][:])
                nc.vector.tensor_copy(out=dst_sb, in_=trp[:])

            def gather_pair(l, t):
                """Gather xl[src] / xr[dst] for all slots of tile t."""
                tg_rows = (t * TILE, (t + 1) * TILE)
                xlg = sp.tile([TILE, KC * HID], F32, tag="xlg")
                xlg3 = xlg[:].rearrange("p (k f) -> p k f", k=KC)
                if KA > 0:
                    ia = mp.tile([TILE, KA * 8], I16, tag="ia")
                    nc.sync.dma_start(
                        out=ia[:], in_=din["gxa"][:, t * KA * 8 : (t + 1) * KA * 8])
                    nc.gpsimd.dma_gather(
                        out_ap=xlg3[:, 0:KA, :], in_ap=xl_full[l][0:HALF, :],
                        idxs_ap=ia[:], num_idxs=KA * TILE, num_idxs_reg=KA * TILE,
                        elem_size=HID, single_packet=False)
                if KB > 0:
                    ib = mp.tile([TILE, KB * 8], I16, tag="ib")
                    nc.sync.dma_start(
                        out=ib[:], in_=din["gxb"][:, t * KB * 8 : (t + 1) * KB * 8])
                    nc.gpsimd.dma_gather(
                        out_ap=xlg3[:, KA:KE, :], in_ap=xl_full[l][HALF:NPAD, :],
                        idxs_ap=ib[:], num_idxs=KB * TILE, num_idxs_reg=KB * TILE,
                        elem_size=HID, single_packet=False)
                # self chunk: own rows from the local slice
                nc.sync.dma_start(
                    out=xlg3[:, KE, :],
                    in_=xl_loc[l][tg_rows[0] : tg_rows[1], :])

                xrg = sp.tile([TILE, KC * HID], F32, tag="xrg")
                xrg3 = xrg[:].rearrange("p (k f) -> p k f", k=KC)
                ir = mp.tile([TILE, KE * 8], I16, tag="ir")
                nc.sync.dma_start(
                    out=ir[:], in_=din["gxr"][:, t * KE * 8 : (t + 1) * KE * 8])
                nc.gpsimd.dma_gather(
                    out_ap=xrg3[:, 0:KE, :], in_ap=xr_loc[l][:, :],
                    idxs_ap=ir[:], num_idxs=KE * TILE, num_idxs_reg=KE * TILE,
                    elem_size=HID, single_packet=False)
                nc.sync.dma_start(
                    out=xrg3[:, KE, :],
                    in_=xr_loc[l][tg_rows[0] : tg_rows[1], :])
                return xlg, xrg

            # ---------------- phase A: node transforms ----------------
            def phase_a(l):
                for t in range(TPC):
                    if l == 0:
                        lhsT = mp.tile([FIN, TILE], F32, tag="hT_in")
                        nc.sync.dma_start(
                            out=lhsT[:], in_=din["xT"][:, t * TILE : (t + 1) * TILE])
                    else:
                        lhsT = mp.tile([HID, TILE], F32, tag="hT_in")
                        nc.sync.dma_start(
                            out=lhsT[:], in_=hT_loc[:, t * TILE : (t + 1) * TILE])
                    for (W, bb, dstd) in (
                        (C[f"Wl{l}"], C[f"blb{l}"], xl_loc[l]),
                        (C[f"Wr{l}"], C[f"brb{l}"], xr_loc[l]),
                    ):
                        ps = pp.tile([TILE, HID], F32, tag="ps")
                        nc.tensor.matmul(out=ps[:], lhsT=lhsT[:], rhs=W[:],
                                         start=True, stop=True)
                        sb = mp.tile([TILE, HID], F32, tag="xout")
                        nc.vector.tensor_add(out=sb[:], in0=ps[:], in1=bb[:])
                        nc.sync.dma_start(
                            out=dstd[t * TILE : (t + 1) * TILE, :], in_=sb[:])

            # ---------------- phase B0: mean edge_attr per node ----------
            def phase_b0():
                for t in range(TPC):
                    w_t = sp.tile([TILE, KC * TILE], F32, tag="w")
                    build_w(t, w_t)
                    em = sp.tile([TILE, KE * (ED + 1)], F32, tag="eaem")
                    nc.sync.dma_start(
                        out=em[:],
                        in_=din["ea_em"][:, t * KE * (ED + 1) : (t + 1) * KE * (ED + 1)])
                    pea = pp.tile([TILE, ED + 1], F32, tag="ps")
                    for c in range(KE):
                        nc.tensor.matmul(
                            out=pea[:],
                            lhsT=w_t[:, c * TILE : (c + 1) * TILE],
                            rhs=em[:, c * (ED + 1) : (c + 1) * (ED + 1)],
                            start=(c == 0), stop=(c == KE - 1))
                    cnt = mp.tile([TILE, 1], F32, tag="cnt")
                    nc.vector.tensor_scalar_max(out=cnt[:], in0=pea[:, ED : ED + 1],
                                                scalar1=1.0)
                    rc = mp.tile([TILE, 1], F32, tag="rc")
                    nc.vector.reciprocal(out=rc[:], in_=cnt[:])
                    la = mp.tile([TILE, ED], F32, tag="la")
                    nc.vector.tensor_scalar_mul(out=la[:], in0=pea[:, 0:ED],
                                                scalar1=rc[:])
                    lt = pp.tile([ED, TILE], F32, tag="ps")
                    nc.tensor.transpose(out=lt[:], in_=la[:], identity=C["ident"][:])
                    lts = mp.tile([ED, TILE], F32, tag="lts")
                    nc.vector.tensor_copy(out=lts[:], in_=lt[:])
                    nc.sync.dma_start(
                        out=laT_loc[:, t * TILE : (t + 1) * TILE], in_=lts[:])

            # ---------------- edge pass ----------------
            def edge_pass(l):
                We = C[f"We{l}"]
                attb = C[f"attb{l}"]
                for t in range(TPC):
                    xlg, xrg = gather_pair(l, t)
                    w_t = sp.tile([TILE, KC * TILE], F32, tag="w")
                    build_w(t, w_t)
                    laT_t = mp.tile([ED, TILE], F32, tag="laT_t")
                    nc.sync.dma_start(
                        out=laT_t[:], in_=laT_loc[:, t * TILE : (t + 1) * TILE])

                    accd = pp.tile([TILE, 8], F32, tag="ps")
                    accn = pp.tile([TILE, HID], F32, tag="ps")

                    for g in range(NGRP):
                        c0 = g * GRP
                        gw = min(GRP, KC - c0)          # group width in chunks
                        ee = pp.tile([TILE, gw * HID], F32, tag="ps")
                        eaT_g = mp.tile([ED, gw * TILE], F32, tag="eaT_g")
                        nc.sync.dma_start(
                            out=eaT_g[:],
                            in_=din["ea_T"][:, (t * KC + c0) * TILE
                                            : (t * KC + c0 + gw) * TILE])
                        for j in range(gw):
                            c = c0 + j
                            lhsT = (laT_t[:] if c == KE
                                    else eaT_g[:, j * TILE : (j + 1) * TILE])
                            nc.tensor.matmul(
                                out=ee[:, j * HID : (j + 1) * HID], lhsT=lhsT,
                                rhs=We[:], start=True, stop=True)
                        mpre_g = mp.tile([TILE, gw * HID], F32, tag="mpre_g")
                        nc.vector.tensor_add(
                            out=mpre_g[:],
                            in0=xlg[:, c0 * HID : (c0 + gw) * HID],
                            in1=xrg[:, c0 * HID : (c0 + gw) * HID])
                        mg = mp.tile([TILE, gw * HID], F32, tag="mg")
                        nc.vector.tensor_add(out=mg[:], in0=mpre_g[:], in1=ee[:])
                        t1 = mp.tile([TILE, gw * HID], F32, tag="t1")
                        nc.vector.tensor_scalar_mul(out=t1[:], in0=mg[:],
                                                    scalar1=SLOPE)
                        nc.vector.tensor_tensor(out=mg[:], in0=mg[:], in1=t1[:],
                                                op=OP.max)
                        nc.vector.tensor_tensor(out=mg[:], in0=mg[:],
                                                in1=attb[:, 0 : gw * HID],
                                                op=OP.mult)
                        s4 = mp.tile([TILE, gw * HEADS], F32, tag="s4")
                        nc.vector.tensor_reduce(
                            out=s4[:],
                            in_=mg[:].rearrange("p (q d) -> p q d", d=D),
                            op=OP.add, axis=AX.X)
                        ex4 = mp.tile([TILE, gw * HEADS], F32, tag="ex4")
                        nc.scalar.activation(out=ex4[:], in_=s4[:], func=AF.Exp,
                                             bias=negb[:, 0:1])
                        rhs = mp.tile([TILE, gw * CW], F32, tag="rhs")
                        rhs3 = rhs[:].rearrange("p (q w) -> p q w", w=CW)
                        nc.vector.tensor_copy(
                            out=rhs3[:, :, 0:8],
                            in_=ex4[:].rearrange("p (q h) -> p q h", h=HEADS))
                        exb = (ex4[:].rearrange("p (q h o) -> p q h o",
                                                h=HEADS, o=1)
                               .to_broadcast([TILE, gw, HEADS, D]))
                        nc.vector.tensor_tensor(
                            out=rhs3[:, :, 8:CW].rearrange(
                                "p q (h d) -> p q h d", d=D),
                            in0=mpre_g[:].rearrange("p (q h d) -> p q h d",
                                                    h=HEADS, d=D),
                            in1=exb, op=OP.mult)
                        for j in range(gw):
                            c = c0 + j
                            nc.tensor.matmul(
                                out=accd[:],
                                lhsT=w_t[:, c * TILE : (c + 1) * TILE],
                                rhs=rhs[:, j * CW : j * CW + 8],
                                start=(c == 0), stop=(c == KC - 1))
                            nc.tensor.matmul(
                                out=accn[:],
                                lhsT=w_t[:, c * TILE : (c + 1) * TILE],
                                rhs=rhs[:, j * CW + 8 : (j + 1) * CW],
                                start=(c == 0), stop=(c == KC - 1))

                    # ---- tile tail ----
                    dens = mp.tile([TILE, 8], F32, tag="dens")
                    nc.vector.tensor_scalar_max(out=dens[:], in0=accd[:],
                                                scalar1=1e-30)
                    rd = mp.tile([TILE, 8], F32, tag="rd")
                    nc.vector.reciprocal(out=rd[:], in_=dens[:])
                    h_t = mp.tile([TILE, HID], F32, tag="h_t")
                    rdb = (rd[:].rearrange("p (h o) -> p h o", o=1)
                           .to_broadcast([TILE, HEADS, D]))
                    nc.vector.tensor_tensor(
                        out=h_t[:].rearrange("p (h d) -> p h d", d=D),
                        in0=accn[:].rearrange("p (h d) -> p h d", d=D),
                        in1=rdb, op=OP.mult)
                    xro = mp.tile([TILE, HID], F32, tag="xro")
                    nc.sync.dma_start(out=xro[:],
                                      in_=xr_loc[l][t * TILE : (t + 1) * TILE, :])
                    nc.vector.tensor_tensor(out=h_t[:], in0=h_t[:], in1=xro[:],
                                            op=OP.subtract)
                    nc.vector.tensor_add(out=h_t[:], in0=h_t[:], in1=C[f"outb{l}"][:])
                    if l < 2:
                        nc.vector.tensor_scalar_max(out=h_t[:], in0=h_t[:],
                                                    scalar1=0.0)
                        hT = mp.tile([HID, TILE], F32, tag="hT_out")
                        transpose_to(hT[:], h_t[:])
                        nc.sync.dma_start(
                            out=hT_loc[:, t * TILE : (t + 1) * TILE], in_=hT[:])
                    else:
                        for r in range(GT):
                            pg = mp.tile([TILE, TILE], F32, tag="pg")
                            bg = (C["batchg"][:, t : t + 1]
                                  .rearrange("p (o q) -> p o q", o=1)
                                  .to_broadcast([TILE, 1, TILE]))
                            ig = (C["iota_g"][:, r * TILE : (r + 1) * TILE]
                                  .rearrange("p (o n) -> p o n", o=1))
                            nc.vector.tensor_tensor(
                                out=pg[:].rearrange("p (o n) -> p o n", o=1),
                                in0=bg, in1=ig, op=OP.is_equal)
                            pps = pp.tile([TILE, HID], F32, tag="ps")
                            nc.tensor.matmul(out=pps[:], lhsT=pg[:], rhs=h_t[:],
                                             start=True, stop=True)
                            if t == 0:
                                nc.vector.tensor_copy(
                                    out=pool_acc[:, r * HID : (r + 1) * HID],
                                    in_=pps[:])
                            else:
                                nc.vector.tensor_add(
                                    out=pool_acc[:, r * HID : (r + 1) * HID],
                                    in0=pool_acc[:, r * HID : (r + 1) * HID],
                                    in1=pps[:])

            # ---------------- MLP tail ----------------
            def mlp_tail():
                for r in range(GT):
                    nc.sync.dma_start(
                        out=pool_part[r * TILE : (r + 1) * TILE, :],
                        in_=pool_acc[:, r * HID : (r + 1) * HID])
                nc.gpsimd.collective_compute(
                    "AllReduce", mybir.AluOpType.add, replica_groups=RG,
                    ins=[pool_part[:, :]], outs=[pool_full[:, :]])
                for r in range(GT):
                    g_sb = mp.tile([TILE, HID], F32, tag="g_sb")
                    nc.sync.dma_start(out=g_sb[:],
                                      in_=pool_full[r * TILE : (r + 1) * TILE, :])
                    gT = mp.tile([HID, TILE], F32, tag="gT")
                    transpose_to(gT[:], g_sb[:])
                    zps = pp.tile([TILE, MH], F32, tag="ps")
                    nc.tensor.matmul(out=zps[:], lhsT=gT[:], rhs=C["mW1"][:],
                                     start=True, stop=True)
                    z = mp.tile([TILE, MH], F32, tag="z")
                    nc.vector.tensor_add(out=z[:], in0=zps[:], in1=C["mb1b"][:])
                    nc.vector.tensor_scalar_max(out=z[:], in0=z[:], scalar1=0.0)
                    mu = mp.tile([TILE, 1], F32, tag="mu")
                    nc.vector.tensor_reduce(out=mu[:], in_=z[:], op=OP.add, axis=AX.X)
                    nc.vector.tensor_scalar_mul(out=mu[:], in0=mu[:],
                                                scalar1=1.0 / MH)
                    nc.vector.tensor_scalar_sub(out=z[:], in0=z[:], scalar1=mu[:])
                    sq = mp.tile([TILE, MH], F32, tag="sq")
                    var = mp.tile([TILE, 1], F32, tag="var")
                    nc.scalar.activation(out=sq[:], in_=z[:], func=AF.Square,
                                         accum_out=var[:])
                    std = mp.tile([TILE, 1], F32, tag="std")
                    nc.scalar.activation(out=std[:], in_=var[:], func=AF.Sqrt,
                                         scale=1.0 / MH, bias=epsb[:, 0:1])
                    rstd = mp.tile([TILE, 1], F32, tag="rstd")
                    nc.vector.reciprocal(out=rstd[:], in_=std[:])
                    nc.vector.tensor_scalar_mul(out=z[:], in0=z[:], scalar1=rstd[:])
                    nc.vector.tensor_tensor(out=z[:], in0=z[:], in1=C["ln_gb"][:],
                                            op=OP.mult)
                    nc.vector.tensor_add(out=z[:], in0=z[:], in1=C["ln_bb"][:])
                    ops = pp.tile([TILE, MO], F32, tag="ps")
                    for k in range(MH // TILE):
                        zT = mp.tile([TILE, TILE], F32, tag="zT")
                        transpose_to(zT[:], z[:, k * TILE : (k + 1) * TILE])
                        nc.tensor.matmul(
                            out=ops[:], lhsT=zT[:],
                            rhs=C["mW2t"][:, k * MO : (k + 1) * MO],
                            start=(k == 0), stop=(k == MH // TILE - 1))
                    o_sb = mp.tile([TILE, MO], F32, tag="o_sb")
                    nc.vector.tensor_add(out=o_sb[:], in0=ops[:], in1=C["mb2b"][:])
                    lo = r * TILE
                    hi = min(cfg.G, lo + TILE)
                    if hi > lo:
                        nc.sync.dma_start(out=out_t[lo:hi, :], in_=o_sb[: hi - lo, :])

            # ---------------- main sequence ----------------
            def tap(nm, src_t):
                if DEBUG_TAPS:
                    nc.sync.dma_start(out=dbg[nm][:, :], in_=src_t[:, :])

            phase_a(0)
            nc.gpsimd.collective_compute(
                "AllGather", mybir.AluOpType.bypass, replica_groups=RG,
                ins=[xl_loc[0][:, :]], outs=[xl_full[0][:, :]])
            tap("d_xl0", xl_loc[0]); tap("d_xr0", xr_loc[0])
            tap("d_xlf0", xl_full[0])
            phase_b0()
            tap("d_laT", laT_loc)
            for l in range(3):
                edge_pass(l)
                if l == 0:
                    tap("d_h0T", hT_loc)
                if l < 2:
                    phase_a(l + 1)
                    nc.gpsimd.collective_compute(
                        "AllGather", mybir.AluOpType.bypass, replica_groups=RG,
                        ins=[xl_loc[l + 1][:, :]], outs=[xl_full[l + 1][:, :]])
            if DEBUG_TAPS:
                for r in range(GT):
                    nc.sync.dma_start(
                        out=dbg["d_pool"][r * TILE : (r + 1) * TILE, :],
                        in_=pool_acc[:, r * HID : (r + 1) * HID])
            mlp_tail()
            tap("d_poolf", pool_full)

    nc.finalize()
    return nc


def make_in_maps(cfg, shared, per_core):
    maps = []
    for c in range(cfg.NC):
        m = dict(shared)
        m.update(per_core[c])
        maps.append(m)
    return maps


def kernel(**inputs) -> np.ndarray:
    from concourse.bass_utils import run_bass_kernel_spmd

    cfg, shared, per_core = prepare(inputs, n_cores=8)
    nc = build(cfg)
    res = run_bass_kernel_spmd(
        nc, make_in_maps(cfg, shared, per_core), core_ids=list(range(8)))
    return res.results[0]["out"]

